# revision 20
# baseline (speedup 1.0000x reference)
"""ClassAttention (decode-style single-query attention) on 8 TRN2 NeuronCores.

Math (per batch b):
    kv = x @ Wkv              # [N, 2*H*D], k half cols 0:1024, v half 1024:2048
    q  = x[0] @ Wq            # [H*D]  (CLS token only)
    logits[t, h] = scale * sum_d q[h,d] * k[t, h*64+d]
    attn = softmax_t(logits)
    cls[h,d] = sum_t attn[t,h] * v[t, h*64+d]
    out = cls @ Wproj + bproj

Key restructuring:
  - k is never materialized: logits = x @ wk_fold, with
    wk_fold[c,h] = scale * sum_d q[h,d] * Wk[c, h*64+d]   (folded per batch).
    This halves the dominant projection matmul (only the v half remains).
  - Softmax is computed without max-subtraction (logits are O(1) by
    construction: x ~ N(0,1), W ~ 0.02*N(0,1) -> |logits| < ~4).
  - Normalization by sum(exp) is deferred past the attention-output matmul
    (it is linear), applied to the 1024-dim cls embedding.

Sharding: pure data-parallel over B: 16 batches / 8 cores = 2 per core.
Weights are replicated; each core returns its [2, 1024] output shard.

Layout notes:
  - The projection contracts over C, so x tiles are transposed to
    [c_partition, t_free] via bf16 X-bar DMA transpose (2-byte dtype req).
  - All matmuls run in bf16 (1 cycle/row on PE) accumulating in fp32 PSUM.
"""

import numpy as np

import concourse.bass as bass
import concourse.mybir as mybir
import concourse.tile as tile
from concourse import bacc
from concourse.bass_utils import run_bass_kernel_spmd
from concourse.masks import make_identity

F32 = mybir.dt.float32
BF16 = mybir.dt.bfloat16

B, SEQ, C = 16, 4096, 1024
H, D = 16, 64
SCALE = D ** -0.5  # 0.125
N_CORES = 8
BPC = B // N_CORES          # batches per core
T_TILES = SEQ // 128        # 32 sequence tiles of 128 rows per batch
CB = C // 128               # 8 contraction blocks


def _build():
    nc = bacc.Bacc(
        "TRN2", target_bir_lowering=False, debug=False, num_devices=N_CORES
    )
    x_ap = nc.dram_tensor("x", [BPC, SEQ, C], F32, kind="ExternalInput").ap()
    wq_ap = nc.dram_tensor("Wq", [C, H * D], F32, kind="ExternalInput").ap()
    wkv_ap = nc.dram_tensor("Wkv", [C, 2 * H * D], F32, kind="ExternalInput").ap()
    wp_ap = nc.dram_tensor("Wproj", [H * D, C], F32, kind="ExternalInput").ap()
    bp_ap = nc.dram_tensor("bproj", [C], F32, kind="ExternalInput").ap()
    out_ap = nc.dram_tensor("out", [BPC, C], F32, kind="ExternalOutput").ap()

    with tile.TileContext(nc) as tc:
        _emit(nc, tc, x_ap, wq_ap, wkv_ap, wp_ap, bp_ap, out_ap)
    nc.compile()
    return nc


def _emit(nc, tc, x_ap, wq_ap, wkv_ap, wp_ap, bp_ap, out_ap):
    with tc.tile_pool(name="consts", bufs=1) as consts:
        # ---- resident bf16 weights: [128, CB*1024], block g = c rows g*128.. ----
        # SWDGE cast-loads f32 -> bf16 directly from DRAM.
        wq_bf = consts.tile([128, CB * 1024], BF16)
        wk_bf = consts.tile([128, CB * 1024], BF16)
        wv_bf = consts.tile([128, CB * 1024], BF16)
        wp_bf = consts.tile([128, CB * 1024], BF16)
        for dst, src_ap, c0 in (
            (wq_bf, wq_ap, 0),
            (wk_bf, wkv_ap, 0),
            (wv_bf, wkv_ap, 1024),
            (wp_bf, wp_ap, 0),
        ):
            for g in range(CB):
                nc.gpsimd.dma_start(
                    dst[:, g * 1024 : (g + 1) * 1024],
                    src_ap[g * 128 : (g + 1) * 128, c0 : c0 + 1024],
                )

        bproj_sb = consts.tile([1, C], F32)
        nc.scalar.dma_start(bproj_sb[:], bp_ap[:].unsqueeze(0))

        # ---- small constant tiles ----
        ones_f32 = consts.tile([128, 1], F32)        # for end-of-batch sums matmul
        nc.vector.memset(ones_f32[:], 1.0)
        sc_row = consts.tile([1, 128], BF16)         # scale * ones: q broadcast
        nc.vector.memset(sc_row[:], SCALE)
        identity = consts.tile([16, 16], F32)        # PE-transpose identity
        make_identity(nc, identity[:])

        with (
            tc.tile_pool(name="xbf", bufs=4) as xbf_pool,
            tc.tile_pool(name="xt", bufs=4) as xt_pool,
            tc.tile_pool(name="xt0", bufs=BPC) as xt0_pool,
            tc.tile_pool(name="vbf", bufs=3) as vbf_pool,
            tc.tile_pool(name="small", bufs=2) as small,
            tc.tile_pool(name="exp", bufs=3) as exp_pool,
        ):
            # Phase A: q + wk_fold for ALL batches upfront (so batch b+1's
            # sweep is never gated on its fold).
            grp0s, wkfs = [], []
            with tc.tile_pool(name="qpsum", bufs=1, space="PSUM") as qpsum:
                for b in range(BPC):
                    grp0 = _load_xt_group(nc, b, 0, x_ap, xt0_pool, xt0_pool,
                                          tag0=f"xbf0_{b}", tag1=f"xt0_{b}")
                    wkf = _emit_qfold(
                        nc, tc, b, qpsum, grp0[1], wq_bf, wk_bf, sc_row, small
                    )
                    grp0s.append(grp0)
                    wkfs.append(wkf)
            # Phase B: sweeps + finalize per batch.
            with tc.tile_pool(name="lgpsum", bufs=2, space="PSUM") as lgpsum:
                for b in range(BPC):
                    _emit_batch(
                        nc, tc, b, x_ap, out_ap,
                        wv_bf, wp_bf, bproj_sb,
                        ones_f32, identity,
                        grp0s[b], wkfs[b],
                        xbf_pool, xt_pool, small, exp_pool,
                        lgpsum,
                    )


GRP = 4  # t-tiles per DMA group (amortizes the ~1us per-DMA issue cost)


def _load_xt_group(nc, b, tg, x_ap, xbf_pool, xt_pool, tag0="xbf", tag1="xt"):
    """Load x rows [tg*512,(tg+1)*512) of batch b as 4 sequence tiles in one
    SWDGE cast-load (f32->bf16), then ONE X-bar transpose for all 4 tiles.
    Result layout: xt[:, (i*CB+g)*128 + t] = x[b, tg*512 + i*128 + t, g*128+p]."""
    x_bf = xbf_pool.tile([128, GRP * C], BF16, tag=tag0, bufs=1 if tag0 != "xbf" else None)
    nc.gpsimd.dma_start(
        x_bf[:].rearrange("p (i c) -> p i c", i=GRP),
        x_ap[b, tg * GRP * 128 : (tg + 1) * GRP * 128, :].rearrange(
            "(i t) c -> t i c", i=GRP
        ),
    )
    xt = xt_pool.tile([128, GRP * CB * 128], BF16, tag=tag1, bufs=1 if tag1 != "xt" else None)
    nc.sync.dma_start_transpose(
        xt[:].rearrange("p (x t) -> p x t", t=128), x_bf[:]
    )
    return x_bf, xt


def _emit_qfold(nc, tc, b, qpsum, xt0, wq_bf, wk_bf, sc_row, small):
    """q = x_cls @ Wq, then wk_fold[c,h] = scale * sum_d q[h,d]*Wk[c,(h,d)]."""
    q_ps = qpsum.tile([1, H * D], F32, tag="qps")
    for g in range(CB):
        lt = xt0[:, g * 128 : g * 128 + 1]  # x_cls^T block g (i=0)
        for ch in range(2):
            nc.tensor.matmul(
                q_ps[0:1, ch * 512 : (ch + 1) * 512],
                lt,
                wq_bf[:, g * 1024 + ch * 512 : g * 1024 + (ch + 1) * 512],
                start=(g == 0),
                stop=(g == CB - 1),
            )
    q_sb = small.tile([1, H * D], BF16, tag="qsb")
    nc.scalar.copy(q_sb[:], q_ps[:])

    # qb[c_p, hd] = scale * q[hd]  (outer product broadcast down partitions)
    qb_ps = qpsum.tile([128, H * D], F32, tag="qbps")
    for ch in range(2):
        nc.tensor.matmul(
            qb_ps[:, ch * 512 : (ch + 1) * 512],
            sc_row[0:1, :],
            q_sb[0:1, ch * 512 : (ch + 1) * 512],
            start=True,
            stop=True,
        )

    wkf_bf = small.tile([128, CB * H], BF16, tag="wkf_bf")
    for g in range(CB):
        prod = small.tile([128, H * D], F32, tag="prod")
        nc.vector.tensor_mul(
            prod[:], wk_bf[:, g * 1024 : (g + 1) * 1024], qb_ps[:]
        )
        wkf_g = small.tile([128, H], F32, tag="wkf_g")
        nc.vector.tensor_reduce(
            wkf_g[:].unsqueeze(2),
            prod[:].rearrange("p (h d) -> p h d", d=D),
            axis=mybir.AxisListType.X,
            op=mybir.AluOpType.add,
        )
        nc.vector.tensor_copy(wkf_bf[:, g * H : (g + 1) * H], wkf_g[:])
    return wkf_bf


def _emit_batch(
    nc, tc, b, x_ap, out_ap,
    wv_bf, wp_bf, bproj_sb,
    ones_f32, identity,
    grp0, wkf_bf,
    xbf_pool, xt_pool, small, exp_pool,
    lgpsum,
):
    """Sweep: logits + softmax + xa = exp^T @ x (reassociated attention);
    then cls = diag(xa_norm @ Wv), out = cls @ Wproj + bproj."""
    with tc.tile_pool(name="xapsum", bufs=1, space="PSUM") as xapsum:
        xa_ps = xapsum.tile([16, 1024], F32, tag="xa")  # sum_t exp[t,h] * x[t,c]

        sums_acc = small.tile([128, GRP * H], F32, tag="sums_acc")
        nc.vector.memset(sums_acc[:], 0.0)

        n_grps = T_TILES // GRP
        for tg in range(n_grps):
            x_bf, xt4 = grp0 if tg == 0 else _load_xt_group(
                nc, b, tg, x_ap, xbf_pool, xt_pool
            )
            # logits for all 4 tiles of the group into one [128, GRP*H] psum,
            # ONE exp + ONE sums-add per group (4x fewer cross-engine hops)
            lg_ps = lgpsum.tile([128, GRP * H], F32, tag="lgps")
            for i in range(GRP):
                for g in range(CB):
                    nc.tensor.matmul(
                        lg_ps[:, i * H : (i + 1) * H],
                        xt4[:, (i * CB + g) * 128 : (i * CB + g + 1) * 128],
                        wkf_bf[:, g * H : (g + 1) * H],
                        start=(g == 0), stop=(g == CB - 1),
                    )
            exp_bf = exp_pool.tile([128, GRP * H], BF16, tag="exp")
            nc.scalar.activation(
                exp_bf[:], lg_ps[:], mybir.ActivationFunctionType.Exp
            )
            nc.vector.tensor_add(sums_acc[:], sums_acc[:], exp_bf[:])
            for i in range(GRP):
                first = tg == 0 and i == 0
                last = tg == n_grps - 1 and i == GRP - 1
                for ch in range(2):
                    nc.tensor.matmul(
                        xa_ps[:, ch * 512 : (ch + 1) * 512],
                        exp_bf[:, i * H : (i + 1) * H],
                        x_bf[:, i * C + ch * 512 : i * C + (ch + 1) * 512],
                        start=first, stop=last,
                    )

        # ---------- finalize ----------
        with tc.tile_pool(name="finpsum", bufs=1, space="PSUM") as finpsum:
            # fold the 4 group lanes, then reduce over partitions via ones-MM
            s64 = small.tile([128, H], F32, tag="s64")
            nc.vector.tensor_add(s64[:], sums_acc[:, 0:H], sums_acc[:, H : 2 * H])
            nc.vector.tensor_add(s64[:], s64[:], sums_acc[:, 2 * H : 3 * H])
            nc.vector.tensor_add(s64[:], s64[:], sums_acc[:, 3 * H : 4 * H])
            sum_ps = lgpsum.tile([1, H], F32, tag="lgps")
            nc.tensor.matmul(
                sum_ps[0:1, :], ones_f32[:, 0:1], s64[:],
                start=True, stop=True,
            )
            rec_sb = small.tile([1, 16], F32, tag="rec")
            nc.vector.reciprocal(rec_sb[:], sum_ps[0:1, :])
            rec_t = small.tile([16, 1], F32, tag="rec_t")
            nc.gpsimd.dma_start(rec_t[:], rec_sb[:])  # [1,16] -> [16,1]

            # normalized per-head x combination, off PSUM
            xa_n = small.tile([16, 1024], F32, tag="xa_n")
            nc.vector.tensor_scalar_mul(xa_n[:], xa_ps[:], rec_t[:])

            # xa^T via PE transpose: xaT[p, g*16+h] = xa_n[h, g*128+p]
            xaT_ps = finpsum.tile([128, 128], F32, tag="fin1")
            for g in range(CB):
                nc.tensor.transpose(
                    xaT_ps[:, g * 16 : (g + 1) * 16],
                    xa_n[:, g * 128 : (g + 1) * 128],
                    identity[0:16, 0:16],
                )
            xaT_bf = small.tile([128, 128], BF16, tag="xaT_bf")
            nc.vector.tensor_copy(xaT_bf[:], xaT_ps[:])

            # cls candidates: cls_ps[h, hd] = sum_c xa_n[h, c] * Wv[c, hd]
            cls_ps = finpsum.tile([16, 1024], F32, tag="fin2")
            for g in range(CB):
                for ch in range(2):
                    nc.tensor.matmul(
                        cls_ps[:, ch * 512 : (ch + 1) * 512],
                        xaT_bf[:, g * 16 : (g + 1) * 16],
                        wv_bf[:, g * 1024 + ch * 512 : g * 1024 + (ch + 1) * 512],
                        start=(g == 0), stop=(g == CB - 1),
                    )
            cls_sb = small.tile([16, 1024], F32, tag="cls_sb")
            nc.vector.tensor_copy(cls_sb[:], cls_ps[:])

            # cls^T via PE transpose, then diagonal pick into [hd%128, hd//128]
            accT_ps = finpsum.tile([128, 128], F32, tag="fin1")
            for g in range(CB):
                nc.tensor.transpose(
                    accT_ps[:, g * 16 : (g + 1) * 16],
                    cls_sb[:, g * 128 : (g + 1) * 128],
                    identity[0:16, 0:16],
                )
            cls_bf = small.tile([128, 8], BF16, tag="cls_bf")
            for h in range(16):
                g, half = h // 2, h % 2
                rows = slice(64 * half, 64 * half + 64)
                nc.vector.tensor_copy(
                    cls_bf[rows, g : g + 1],
                    accT_ps[rows, g * 16 + h : g * 16 + h + 1],
                )

            # out = cls @ Wproj + bproj
            o_ps = finpsum.tile([1, C], F32, tag="fin2")
            for g in range(CB):
                for ch in range(2):
                    nc.tensor.matmul(
                        o_ps[0:1, ch * 512 : (ch + 1) * 512],
                        cls_bf[:, g : g + 1],
                        wp_bf[:, g * 1024 + ch * 512 : g * 1024 + (ch + 1) * 512],
                        start=(g == 0), stop=(g == CB - 1),
                    )
            o_sb = small.tile([1, C], F32, tag="osb")
            nc.vector.tensor_add(o_sb[:], o_ps[:], bproj_sb[:])
            nc.scalar.dma_start(out_ap[b : b + 1, :], o_sb[:])


_CACHED = None


def _get_program():
    global _CACHED
    if _CACHED is None:
        _CACHED = _build()
    return _CACHED


def kernel(x, Wq, Wkv, Wproj, bproj, _trace=False):
    x = np.ascontiguousarray(np.asarray(x, dtype=np.float32))
    Wq = np.ascontiguousarray(np.asarray(Wq, dtype=np.float32))
    Wkv = np.ascontiguousarray(np.asarray(Wkv, dtype=np.float32))
    Wproj = np.ascontiguousarray(np.asarray(Wproj, dtype=np.float32))
    bproj = np.ascontiguousarray(np.asarray(bproj, dtype=np.float32))

    nc = _get_program()
    in_maps = [
        {
            "x": x[cid * BPC : (cid + 1) * BPC],
            "Wq": Wq,
            "Wkv": Wkv,
            "Wproj": Wproj,
            "bproj": bproj,
        }
        for cid in range(N_CORES)
    ]
    res = run_bass_kernel_spmd(
        nc, in_maps, core_ids=list(range(N_CORES)), trace=_trace
    )
    out = np.concatenate([res.results[cid]["out"] for cid in range(N_CORES)], axis=0)
    if _trace:
        kernel.last_exec_time_ns = res.exec_time_ns
        kernel.last_results = res
    return out.reshape(B, 1, C)
